# revision 1
# baseline (speedup 1.0000x reference)
"""Multi-head attention Bass/Tile kernel for Trainium2, sharded over 8 NeuronCores.

Full MHA: Q/K/V projections + softmax attention + output projection.
Sharding: core c handles batch b=c//2 and head-group g=c%2 (8 of 16 heads).
Each core returns a partial output [S, D]; the host sums the two head-group
partials per batch and adds the output bias.

Per-core dataflow (all matmul inputs bf16, accumulation fp32 in PSUM):
  QT[f,s] = wqT.T @ xqT          (f on partitions -> scores lhsT/rhs layout)
  KT[f,s] = wkT.T @ xkT
  V[s,f]  = xvT.T @ wvT          (s on partitions -> ctx lhsT layout)
  per (q-chunk, head-pair, k-tile):
    ST[k, q]    = KT_h.T @ QT_h  (row-tiled pair: heads at PE rows 0-63/64-127)
    PT          = exp(ST/8)      (ScalarE, one N=1024 instr for both heads)
    ctxT'[c,q] += V'_h.T @ PT_h  (V' = [V_h | 1], M=65: row 64 of the psum
                                  tile accumulates the softmax row-sum free)
  ctxT = ctxT' * recip(row 64)   (DVE recip -> one base-0 GpSimd partition
                                  broadcast -> DVE mul during eviction)
  out[s,j]    = ctxT.T @ woT     (partial; host adds the two groups + bo)
  Projections and out-projections are emitted as deadline-ordered filler
  between attention k-steps; the Tile scheduler packs them into ACT slack.

No score-max subtraction: inputs are unit-variance gaussians through scaled
projections, so scores are ~N(0,1); exp never overflows.
"""

import numpy as np
import ml_dtypes

BF16 = ml_dtypes.bfloat16

# Full-problem constants (hardcoded; kernel.py must be self-contained).
FULL = dict(S=2048, D=1024, G=8, QC=512)
N_CORES = 8
DH = 64


def build_body(nc, S, D, G, QC, repeat=1):
    """Emit the per-core kernel body onto `nc`. Parameterized so a mini config
    can be CoreSim'd quickly; production uses FULL."""
    import concourse.tile as tile
    from concourse import mybir
    from contextlib import ExitStack

    GF = G * DH
    KT_N = S // 128          # k tiles
    DT_N = D // 128          # d tiles
    FT_N = GF // 128         # f tiles = head pairs
    QC_N = S // QC           # q chunks
    SUB = QC // 128          # s subtiles per q chunk
    JW = min(512, D)         # out-proj j width
    J_N = D // JW            # j chunks
    SC_W = min(512, S)       # proj s-chunk width
    SC_N = S // SC_W
    f32 = mybir.dt.float32
    bf16 = mybir.dt.bfloat16
    EXP = mybir.ActivationFunctionType.Exp

    xqT = nc.dram_tensor("xqT", [D, S], bf16, kind="ExternalInput").ap()
    xkT = nc.dram_tensor("xkT", [D, S], bf16, kind="ExternalInput").ap()
    xvT = nc.dram_tensor("xvT", [D, S], bf16, kind="ExternalInput").ap()
    wqT = nc.dram_tensor("wqT", [D, GF], bf16, kind="ExternalInput").ap()
    wkT = nc.dram_tensor("wkT", [D, GF], bf16, kind="ExternalInput").ap()
    wvT = nc.dram_tensor("wvT", [D, GF], bf16, kind="ExternalInput").ap()
    woT = nc.dram_tensor("woT", [GF, D], bf16, kind="ExternalInput").ap()
    bq_d = nc.dram_tensor("bq", [FT_N, 128], f32, kind="ExternalInput").ap()
    bk_d = nc.dram_tensor("bk", [FT_N, 128], f32, kind="ExternalInput").ap()
    bv_d = nc.dram_tensor("bv", [1, GF], f32, kind="ExternalInput").ap()
    out_d = nc.dram_tensor("out", [S, D], f32, kind="ExternalOutput").ap()

    with tile.TileContext(nc) as tc, ExitStack() as ctx:
        pw = ctx.enter_context(tc.tile_pool(name="pw", bufs=3 * DT_N))
        pwo = ctx.enter_context(tc.tile_pool(name="pwo", bufs=FT_N))
        px = ctx.enter_context(tc.tile_pool(name="px", bufs=2 * DT_N))
        pqt = ctx.enter_context(tc.tile_pool(name="pqt", bufs=FT_N))
        pkt = ctx.enter_context(tc.tile_pool(name="pkt", bufs=FT_N))
        pv = ctx.enter_context(tc.tile_pool(name="pv", bufs=KT_N))
        ppt = ctx.enter_context(tc.tile_pool(name="ppt", bufs=3))
        pctx = ctx.enter_context(tc.tile_pool(name="pctx", bufs=FT_N))
        psm = ctx.enter_context(tc.tile_pool(name="psm", bufs=2))
        pout = ctx.enter_context(tc.tile_pool(name="pout", bufs=3))
        pcst = ctx.enter_context(tc.tile_pool(name="pcst", bufs=1))
        # PSUM pools: ST 2x2 banks + ctx 2 + rowsum 1 + proj/out 1 = 8 banks
        pst_ps = ctx.enter_context(tc.tile_pool(name="pst_ps", bufs=2, space="PSUM"))
        pctx_ps = ctx.enter_context(tc.tile_pool(name="pctx_ps", bufs=2, space="PSUM"))
        pmm_ps = ctx.enter_context(tc.tile_pool(name="pmm_ps", bufs=2, space="PSUM"))

        # ---- constants / weights. wv + bv load upfront (V projection runs
        # first); the rest is emitted after the first xvT DMAs so the V-path
        # loads win the DMA queues at startup. ----
        wq_sb, wk_sb, wv_sb = [], [], []
        wo_sb = []
        bq_sb, bk_sb = [], []
        for dt in range(DT_N):
            t = pw.tile([128, GF], bf16, tag="w", name=f"wv{dt}")
            nc.sync.dma_start(t[:], wvT[dt * 128:(dt + 1) * 128, :])
            wv_sb.append(t)
        bv_row = pcst.tile([1, GF], f32, tag="bvrow")
        nc.sync.dma_start(bv_row[:], bv_d[:])
        bvb = pcst.tile([128, GF], f32, tag="bvb")
        nc.gpsimd.partition_broadcast(bvb[:], bv_row[0:1, :])
        # warm the exp table during the projection phase
        warm = pcst.tile([1, 8], f32, tag="warm")
        nc.vector.memset(warm[:], 0.0)
        nc.scalar.activation(warm[:], warm[:], EXP)

        def load_rest_weights():
            if wq_sb:
                return
            for dt in range(DT_N):
                for lst, src, nm in ((wq_sb, wqT, "wq"), (wk_sb, wkT, "wk")):
                    t = pw.tile([128, GF], bf16, tag="w", name=f"{nm}{dt}")
                    nc.sync.dma_start(t[:], src[dt * 128:(dt + 1) * 128, :])
                    lst.append(t)
            for ft in range(FT_N):
                t = pwo.tile([128, D], bf16, tag="wo")
                nc.sync.dma_start(t[:], woT[ft * 128:(ft + 1) * 128, :])
                wo_sb.append(t)
            for ft in range(FT_N):
                for lst, src in ((bq_sb, bq_d), (bk_sb, bk_d)):
                    t = pcst.tile([128, 1], f32, tag="bias", bufs=2 * FT_N)
                    nc.sync.dma_start(t[:], src[ft:ft + 1, :].rearrange("a b -> b a"))
                    lst.append(t)

        # ---- repeated body (repeat>1 only for steady-state benchmarking) ----
        for _rep in range(repeat):
            _emit_rep(nc, tc, locals())
    return nc


def _emit_rep(nc, tc, env):
    """One full projection+attention+out-projection pass."""
    from concourse import mybir
    f32 = mybir.dt.float32
    bf16 = mybir.dt.bfloat16
    EXP = mybir.ActivationFunctionType.Exp
    S, D, G, QC = env["S"], env["D"], env["G"], env["QC"]
    DT_N, FT_N, KT_N, QC_N, SUB = (
        env["DT_N"], env["FT_N"], env["KT_N"], env["QC_N"], env["SUB"])
    SC_W, SC_N, JW, J_N, GF = env["SC_W"], env["SC_N"], env["JW"], env["J_N"], env["GF"]
    xqT, xkT, xvT, out_d = env["xqT"], env["xkT"], env["xvT"], env["out_d"]
    wq_sb, wk_sb, wv_sb, wo_sb = env["wq_sb"], env["wk_sb"], env["wv_sb"], env["wo_sb"]
    bq_sb, bk_sb, bvb = env["bq_sb"], env["bk_sb"], env["bvb"]
    load_rest_weights = env["load_rest_weights"]
    px, pqt, pkt, pv, ppt, pctx, psm, pout = (
        env["px"], env["pqt"], env["pkt"], env["pv"], env["ppt"],
        env["pctx"], env["psm"], env["pout"])
    pst_ps, pctx_ps, pmm_ps = (env["pst_ps"], env["pctx_ps"], env["pmm_ps"])

    if True:
        # ---- V projection (V[s, f] layout) ----
        xv_sb = []
        for dt in range(DT_N):
            t = px.tile([128, S], bf16, tag="x")
            nc.sync.dma_start(t[:], xvT[dt * 128:(dt + 1) * 128, :])
            xv_sb.append(t)
        load_rest_weights()
        # V stored with a ones column appended per head ([V_h | 1] , 65 cols
        # per head) so the context matmul's 65th output row is the softmax
        # row-sum -- no separate rowsum matmuls needed.
        v_sb = []
        for st in range(KT_N):
            ps = pmm_ps.tile([128, GF], f32, tag="mm")
            for dt in range(DT_N):
                nc.tensor.matmul(
                    ps[:], xv_sb[dt][:, st * 128:(st + 1) * 128], wv_sb[dt][:],
                    start=(dt == 0), stop=(dt == DT_N - 1))
            t = pv.tile([128, G * 65], bf16, tag="v")
            tv = t[:].rearrange("p (g e) -> p g e", e=65)
            nc.vector.tensor_add(
                tv[:, :, 0:64], ps[:].rearrange("p (g d) -> p g d", d=64),
                bvb[:].rearrange("p (g d) -> p g d", d=64))
            nc.vector.memset(tv[:, :, 64:65], 1.0)
            v_sb.append(t)

        # ---- Q/K projections (QT/KT [f, s] layout). The first f-tile is
        # emitted upfront; the rest become fine-grained filler closures popped
        # between attention k-steps so the PE never bunches projection work
        # while ACT (the bottleneck engine) starves. ----
        from collections import deque
        qt_sb = [pqt.tile([128, S], bf16, tag="q", name=f"qt{i}") for i in range(FT_N)]
        kt_sb = [pkt.tile([128, S], bf16, tag="k", name=f"kt{i}") for i in range(FT_N)]
        x_sb = {}

        def load_x(name, src):
            if name not in x_sb:
                tiles = []
                for dt in range(DT_N):
                    t = px.tile([128, S], bf16, tag="x")
                    nc.sync.dma_start(t[:], src[dt * 128:(dt + 1) * 128, :])
                    tiles.append(t)
                x_sb[name] = tiles

        def proj_group(name, wlist, blist, dst, ft, sc):
            out_t = dst[ft]
            ps = pmm_ps.tile([128, SC_W], f32, tag="mm")
            for dt in range(DT_N):
                nc.tensor.matmul(
                    ps[:], wlist[dt][:, ft * 128:(ft + 1) * 128],
                    x_sb[name][dt][:, sc * SC_W:(sc + 1) * SC_W],
                    start=(dt == 0), stop=(dt == DT_N - 1))
            nc.vector.tensor_scalar_add(
                out_t[:, sc * SC_W:(sc + 1) * SC_W], ps[:], blist[ft][:])

        load_x("q", xqT)
        load_x("k", xkT)
        for sc in range(SC_N):
            proj_group("q", wq_sb, bq_sb, qt_sb, 0, sc)
            proj_group("k", wk_sb, bk_sb, kt_sb, 0, sc)

        # Filler queue in DEADLINE order. Block (qc, ft) reads ALL of kt[ft]
        # (scores sweep the full k range) but only the qc chunk of qt[ft], so
        # per ft the 4 K-groups + Q-group(sc=0) must be emitted before the
        # block; later Q-groups are due only at their own q-chunk. Tile
        # derives deps from emission order — a read emitted before its writer
        # gets no dependency and races on HW, so deadlines are hard.
        def q_item(ft, sc):
            return lambda: proj_group("q", wq_sb, bq_sb, qt_sb, ft, sc)

        def k_item(ft, sc):
            return lambda: proj_group("k", wk_sb, bk_sb, kt_sb, ft, sc)

        pe_filler = deque()
        for ft in range(1, FT_N):
            for sc in range(SC_N):
                pe_filler.append(k_item(ft, sc))
            pe_filler.append(q_item(ft, 0))
        for sc in range(1, SC_N):
            for ft in range(1, FT_N):
                pe_filler.append(q_item(ft, sc))

        ctx_sb = [None] * FT_N

        def ctx_mm(ctx_h, ft, kt, pt):
            for h in range(2):
                h65 = (2 * ft + h) * 65
                nc.tensor.matmul(
                    ctx_h[h][:, :], v_sb[kt][:, h65:h65 + 65],
                    pt[:, h * QC:(h + 1) * QC],
                    start=(kt == 0), stop=(kt == KT_N - 1),
                    skip_group_check=True)

        def outproj_group(qc, su, j):
            rows = slice(qc * QC + su * 128, qc * QC + (su + 1) * 128)
            ps = pmm_ps.tile([128, JW], f32, tag="mm")
            for ft in range(FT_N):
                nc.tensor.matmul(
                    ps[:], ctx_sb[ft][:, rows], wo_sb[ft][:, j * JW:(j + 1) * JW],
                    start=(ft == 0), stop=(ft == FT_N - 1))
            o = pout.tile([128, JW], f32, tag="o")
            nc.vector.tensor_copy(o[:], ps[:])
            nc.sync.dma_start(out_d[rows, j * JW:(j + 1) * JW], o[:])

        # ---- attention, q-chunk major; proj/out-proj filler interleaved ----
        for qc in range(QC_N):
            qs = slice(qc * QC, (qc + 1) * QC)
            for ft in range(FT_N):
                if ctx_sb[ft] is None:
                    ctx_sb[ft] = pctx.tile([128, S], bf16, tag="ctx", name=f"ctxsb{ft}")
                ctx_h = [pctx_ps.tile([65, QC], f32, tag="ctx", name=f"ctxps{h}")
                         for h in range(2)]
                pt_tiles = []
                pop_mod = 3 if qc == 0 else 8
                for kt in range(KT_N):
                    if kt % pop_mod == 0 and pe_filler:
                        pe_filler.popleft()()
                    st_ps = pst_ps.tile([128, 2 * QC], f32, tag="st")
                    ks = slice(kt * 128, (kt + 1) * 128)
                    for h in range(2):
                        hp = slice(h * 64, (h + 1) * 64)
                        nc.tensor.matmul(
                            st_ps[:, h * QC:(h + 1) * QC],
                            kt_sb[ft][hp, ks], qt_sb[ft][hp, qs],
                            start=True, stop=True, tile_position=(h * 64, 0))
                    pt = ppt.tile([128, 2 * QC], bf16, tag="pt")
                    nc.scalar.activation(pt[:], st_ps[:], EXP, scale=0.125)
                    pt_tiles.append(pt)
                    # ctx matmuls run one k-step behind the scores so the PE
                    # never queues a PT-dependent matmul ahead of the next
                    # scores pair (keeps ACT fed back-to-back)
                    if kt >= 1:
                        ctx_mm(ctx_h, ft, kt - 1, pt_tiles[kt - 1])
                ctx_mm(ctx_h, ft, KT_N - 1, pt_tiles[KT_N - 1])
                # evict raw ctx+rowsum to SBUF fp32 immediately (frees the
                # PSUM slot for the next block), then normalize from SBUF:
                # recip row 64, DMA both inv rows into partition 0, one base-0
                # partition broadcast, multiply.
                ctxr = [psm.tile([65, QC], f32, tag="ctxr", bufs=4,
                                 name=f"ctxr{h}") for h in range(2)]
                for h in range(2):
                    nc.vector.tensor_copy(ctxr[h][:, :], ctx_h[h][:, :])
                invt = psm.tile([65, 2 * QC], f32, tag="invt", bufs=1)
                invc = psm.tile([1, 2 * QC], f32, tag="invc")
                for h in range(2):
                    nc.vector.reciprocal(
                        invt[64:65, h * QC:(h + 1) * QC], ctxr[h][64:65, :])
                    nc.sync.dma_start(
                        invc[0:1, h * QC:(h + 1) * QC],
                        invt[64:65, h * QC:(h + 1) * QC])
                bc = psm.tile([128, 2 * QC], f32, tag="bc", bufs=1)
                nc.gpsimd.partition_broadcast(bc[:, :], invc[0:1, :])
                nc.vector.tensor_mul(
                    ctx_sb[ft][0:64, qs], ctxr[0][0:64, :], bc[0:64, 0:QC])
                oddt = psm.tile([64, QC], bf16, tag="oddt")
                nc.vector.tensor_mul(
                    oddt[:, :], ctxr[1][0:64, :], bc[0:64, QC:2 * QC])
                nc.sync.dma_start(ctx_sb[ft][64:128, qs], oddt[:, :])
            # queue this q-chunk's out-projection as filler for later blocks
            for su in range(SUB):
                for j in range(J_N):
                    pe_filler.append(
                        lambda qc=qc, su=su, j=j: outproj_group(qc, su, j))
        while pe_filler:
            pe_filler.popleft()()
    return nc


def build_nc(S=None, D=None, G=None, QC=None, num_devices=N_CORES, repeat=1):
    cfg = dict(FULL)
    for k, v in (("S", S), ("D", D), ("G", G), ("QC", QC)):
        if v is not None:
            cfg[k] = v
    from concourse import bacc
    nc = bacc.Bacc("TRN2", target_bir_lowering=False, debug=False,
                   num_devices=num_devices)
    build_body(nc, **cfg, repeat=repeat)
    nc.compile()
    return nc


def shard_inputs(q, k, v, Wq, bq, Wk, bk, Wv, bv, Wo, bo,
                 S=None, D=None, G=None, n_cores=N_CORES):
    """Build the per-core input maps (host-side shard + transpose + bf16 cast)."""
    S = S or FULL["S"]
    D = D or FULL["D"]
    G = G or FULL["G"]
    GF = G * DH
    FT_N = GF // 128
    n_groups = (q.shape[2] // DH * DH // GF) if False else (Wq.shape[0] // GF)
    in_maps = []
    for c in range(n_cores):
        b, g = divmod(c, n_groups)
        gs = slice(g * GF, (g + 1) * GF)
        m = {
            "xqT": np.ascontiguousarray(q[b].T).astype(BF16),
            "xkT": np.ascontiguousarray(k[b].T).astype(BF16),
            "xvT": np.ascontiguousarray(v[b].T).astype(BF16),
            "wqT": np.ascontiguousarray(Wq[gs, :].T).astype(BF16),
            "wkT": np.ascontiguousarray(Wk[gs, :].T).astype(BF16),
            "wvT": np.ascontiguousarray(Wv[gs, :].T).astype(BF16),
            "woT": np.ascontiguousarray(Wo[:, gs].T).astype(BF16),
            "bq": np.ascontiguousarray(bq[gs]).reshape(FT_N, 128).astype(np.float32),
            "bk": np.ascontiguousarray(bk[gs]).reshape(FT_N, 128).astype(np.float32),
            "bv": np.ascontiguousarray(bv[gs]).reshape(1, GF).astype(np.float32),
        }
        in_maps.append(m)
    return in_maps


def gather_outputs(results, bo, n_groups=2):
    """Sum head-group partials per batch and add the output bias."""
    n_b = len(results) // n_groups
    outs = []
    for b in range(n_b):
        acc = results[b * n_groups]["out"].astype(np.float32)
        for g in range(1, n_groups):
            acc = acc + results[b * n_groups + g]["out"]
        outs.append(acc + np.asarray(bo, np.float32)[None, :])
    return np.stack(outs, axis=0)


_NC_CACHE = {}


def kernel(q, k, v, Wq, bq, Wk, bk, Wv, bv, Wo, bo):
    from concourse.bass_utils import run_bass_kernel_spmd
    key = "full"
    if key not in _NC_CACHE:
        _NC_CACHE[key] = build_nc()
    nc = _NC_CACHE[key]
    in_maps = shard_inputs(q, k, v, Wq, bq, Wk, bk, Wv, bv, Wo, bo)
    res = run_bass_kernel_spmd(nc, in_maps, core_ids=list(range(N_CORES)))
    return gather_outputs(res.results, bo)



# revision 5
# speedup vs baseline: 3.5156x; 3.5156x over previous
"""Multi-head attention Bass/Tile kernel for Trainium2, sharded over 8 NeuronCores.

Full MHA: Q/K/V projections + softmax attention + output projection.
Sharding: core c handles batch b=c//2 and head-group g=c%2 (8 of 16 heads).
Each core returns a partial output [S, D] (bf16); the host sums the two
head-group partials per batch and adds the output bias.

Per-core dataflow (matmul inputs bf16, accumulation fp32 in PSUM):
  V[s,f]  = xvT.T @ wvT            (s on partitions -> ctx lhsT layout)
  QT[f,s] = wqT.T @ xqT            (f on partitions -> scores lhsT/rhs layout)
  KT[f,s] = wkT.T @ xkT
  per (q-chunk, head-pair, k-tile):
    ST[k, q]    = KT_h.T @ QT_h    (row-tiled pair: heads at PE rows 0-63/64-127)
    PT          = exp(ST/8)        (ScalarE, one N=1024 instr for both heads)
    ctxT'[c,q] += V'_h.T @ PT_h    (V' = [V_h | 1], M=65: row 64 of the psum
                                    tile accumulates the softmax row-sum free)
  normalize: one bf16 PSUM->SBUF eviction per head, one DVE recip (64-down
  partition shift to p0), one GpSimd partition broadcast, two 2x-mode DVE
  muls (inputs base-0; odd head written with a +64 output partition shift —
  HW requires SBUF inputs to share a base partition, outputs need not).
  out[s,j]    = ctxT.T @ woT       (partial, bf16; host adds groups + bo)

DMA strategy: the HWDGE queue costs ~625ns per DMA instruction (serial), so
loads are merged: one DMA per weight tensor (dt-blocks side by side in the
free dim of one wide tile, addressed back via views) and one DMA per
(x-tensor, 512-column s-chunk). xk streams before xq; V projection and the
ft0 K/Q projections consume chunks as they land, so the first attention
block starts ~20us in instead of waiting for all 15.5MB of input DMA.
Remaining projections and out-projections are deadline-tracked filler popped
between attention k-steps; a due-check before every k-step guarantees a
filler write is emitted before its first reader (Tile derives deps from
emission order; a read emitted before its writer gets no dependency and
races on HW).

No score-max subtraction: inputs are unit-variance gaussians through scaled
projections, so scores are ~N(0,1); exp never overflows.
"""

import numpy as np
import ml_dtypes

BF16 = ml_dtypes.bfloat16

# Full-problem constants (hardcoded; kernel.py must be self-contained).
FULL = dict(S=2048, D=1024, G=8, QC=512)
N_CORES = 8
DH = 64


def build_body(nc, S, D, G, QC, repeat=1):
    """Emit the per-core kernel body onto `nc`. Parameterized so a mini config
    can be CoreSim'd quickly; production uses FULL."""
    import concourse.tile as tile
    from concourse import mybir
    from contextlib import ExitStack

    GF = G * DH
    KT_N = S // 128          # k tiles
    DT_N = D // 128          # d tiles
    FT_N = GF // 128         # f tiles = head pairs
    QC_N = S // QC           # q chunks
    SUB = QC // 128          # s subtiles per q chunk
    JW = min(512, D)         # out-proj j width
    J_N = D // JW            # j chunks
    SC_W = min(512, S)       # proj s-chunk width
    SC_N = S // SC_W
    f32 = mybir.dt.float32
    bf16 = mybir.dt.bfloat16
    EXP = mybir.ActivationFunctionType.Exp

    xqT = nc.dram_tensor("xqT", [D, S], bf16, kind="ExternalInput").ap()
    xkT = nc.dram_tensor("xkT", [D, S], bf16, kind="ExternalInput").ap()
    xvT = nc.dram_tensor("xvT", [D, S], bf16, kind="ExternalInput").ap()
    wqT = nc.dram_tensor("wqT", [D, GF], bf16, kind="ExternalInput").ap()
    wkT = nc.dram_tensor("wkT", [D, GF], bf16, kind="ExternalInput").ap()
    wvT = nc.dram_tensor("wvT", [D, GF], bf16, kind="ExternalInput").ap()
    woT = nc.dram_tensor("woT", [GF, D], bf16, kind="ExternalInput").ap()
    bq_d = nc.dram_tensor("bq", [FT_N, 128], f32, kind="ExternalInput").ap()
    bk_d = nc.dram_tensor("bk", [FT_N, 128], f32, kind="ExternalInput").ap()
    bv_d = nc.dram_tensor("bv", [1, GF], f32, kind="ExternalInput").ap()
    out_d = nc.dram_tensor("out", [S, D], bf16, kind="ExternalOutput").ap()

    with tile.TileContext(nc) as tc, ExitStack() as ctx:
        pw = ctx.enter_context(tc.tile_pool(name="pw", bufs=3))
        pwo = ctx.enter_context(tc.tile_pool(name="pwo", bufs=1))
        pxv = ctx.enter_context(tc.tile_pool(name="pxv", bufs=2))
        px = ctx.enter_context(tc.tile_pool(name="px", bufs=2))
        pqt = ctx.enter_context(tc.tile_pool(name="pqt", bufs=FT_N))
        pkt = ctx.enter_context(tc.tile_pool(name="pkt", bufs=FT_N))
        pv = ctx.enter_context(tc.tile_pool(name="pv", bufs=KT_N))
        ppt = ctx.enter_context(tc.tile_pool(name="ppt", bufs=3))
        pctx = ctx.enter_context(tc.tile_pool(name="pctx", bufs=FT_N))
        psm = ctx.enter_context(tc.tile_pool(name="psm", bufs=2))
        pout = ctx.enter_context(tc.tile_pool(name="pout", bufs=2))
        pcst = ctx.enter_context(tc.tile_pool(name="pcst", bufs=1))
        # PSUM banks: ST 2x2 + ctx 2x1 + proj/out 2x1 = 8
        pst_ps = ctx.enter_context(tc.tile_pool(name="pst_ps", bufs=2, space="PSUM"))
        pctx_ps = ctx.enter_context(tc.tile_pool(name="pctx_ps", bufs=2, space="PSUM"))
        pmm_ps = ctx.enter_context(tc.tile_pool(name="pmm_ps", bufs=2, space="PSUM"))

        # ---- wv + bv load upfront (V projection runs first); wq/wk/biases
        # are emitted after the first xv chunk DMA, wo after the x DMAs.
        # Each weight tensor is ONE wide tile ([p, dt-blocks]) = one DMA. ----
        wq_sb, wk_sb, wv_sb = [], [], []
        wo_sb = []
        holder = {}

        def load_w(name, src, lst, ncols):
            t = pw.tile([128, DT_N * ncols], bf16, tag="w", name=name)
            nc.sync.dma_start(
                t[:].rearrange("p (dt f) -> p dt f", dt=DT_N),
                src.rearrange("(dt p) f -> p dt f", p=128))
            lst.extend(t[:, dt * ncols:(dt + 1) * ncols] for dt in range(DT_N))

        load_w("wv", wvT, wv_sb, GF)
        bv_row = pcst.tile([1, GF], f32, tag="bvrow")
        nc.sync.dma_start(bv_row[:], bv_d[:])
        bvb = pcst.tile([128, GF], f32, tag="bvb")
        nc.gpsimd.partition_broadcast(bvb[:], bv_row[0:1, :])
        # warm the exp table during the projection phase
        warm = pcst.tile([1, 8], f32, tag="warm")
        nc.vector.memset(warm[:], 0.0)
        nc.scalar.activation(warm[:], warm[:], EXP)

        def load_w_qk():
            if wq_sb:
                return
            load_w("wq", wqT, wq_sb, GF)
            load_w("wk", wkT, wk_sb, GF)
            bias_q = pcst.tile([128, FT_N], f32, tag="biasq")
            bias_k = pcst.tile([128, FT_N], f32, tag="biask")
            nc.sync.dma_start(bias_q[:], bq_d.rearrange("a b -> b a"))
            nc.sync.dma_start(bias_k[:], bk_d.rearrange("a b -> b a"))
            holder["bq"] = [bias_q[:, ft:ft + 1] for ft in range(FT_N)]
            holder["bk"] = [bias_k[:, ft:ft + 1] for ft in range(FT_N)]

        def load_wo():
            if wo_sb:
                return
            t = pwo.tile([128, FT_N * D], bf16, tag="wo")
            nc.sync.dma_start(
                t[:].rearrange("p (ft f) -> p ft f", ft=FT_N),
                woT.rearrange("(ft p) f -> p ft f", p=128))
            wo_sb.extend(t[:, ft * D:(ft + 1) * D] for ft in range(FT_N))

        # ---- repeated body (repeat>1 only for steady-state benchmarking) ----
        for _rep in range(repeat):
            _emit_rep(nc, tc, locals())
    return nc


def _emit_rep(nc, tc, env):
    """One full projection+attention+out-projection pass."""
    from collections import deque
    from concourse import mybir
    f32 = mybir.dt.float32
    bf16 = mybir.dt.bfloat16
    EXP = mybir.ActivationFunctionType.Exp
    S, D, G, QC = env["S"], env["D"], env["G"], env["QC"]
    DT_N, FT_N, KT_N, QC_N, SUB = (
        env["DT_N"], env["FT_N"], env["KT_N"], env["QC_N"], env["SUB"])
    SC_W, SC_N, JW, J_N, GF = env["SC_W"], env["SC_N"], env["JW"], env["J_N"], env["GF"]
    xqT, xkT, xvT, out_d = env["xqT"], env["xkT"], env["xvT"], env["out_d"]
    wq_sb, wk_sb, wv_sb, wo_sb = env["wq_sb"], env["wk_sb"], env["wv_sb"], env["wo_sb"]
    bvb, holder = env["bvb"], env["holder"]
    load_w_qk, load_wo = env["load_w_qk"], env["load_wo"]
    pxv, px, pqt, pkt, pv, ppt, pctx, psm, pout = (
        env["pxv"], env["px"], env["pqt"], env["pkt"], env["pv"], env["ppt"],
        env["pctx"], env["psm"], env["pout"])
    pst_ps, pctx_ps, pmm_ps = (env["pst_ps"], env["pctx_ps"], env["pmm_ps"])
    ST_PER_SC = SC_W // 128

    def dma_x_chunk(dst_all, src, sc):
        """One DMA: all dt-blocks of one 512-col s-chunk."""
        scs = slice(sc * SC_W, (sc + 1) * SC_W)
        dst = dst_all[:].rearrange("p (dt s) -> p dt s", s=S)[:, :, scs]
        nc.sync.dma_start(
            dst, src[:, scs].rearrange("(dt p) f -> p dt f", p=128))

    # ---- V projection, s-chunk pipelined: one xv DMA per s-chunk; each
    # group of s-tiles only needs its own chunk. V is stored with a ones
    # column appended per head ([V_h | 1], 65 cols per head) so the context
    # matmul's 65th output row is the softmax row-sum for free. ----
    v_sb = []
    xk_all = px.tile([128, DT_N * S], bf16, tag="x", name="xk")
    xq_all = px.tile([128, DT_N * S], bf16, tag="x", name="xq")
    for sc in range(SC_N):
        xv_all = pxv.tile([128, DT_N * SC_W], bf16, tag="xv")
        scs = slice(sc * SC_W, (sc + 1) * SC_W)
        nc.sync.dma_start(
            xv_all[:].rearrange("p (dt f) -> p dt f", dt=DT_N),
            xvT[:, scs].rearrange("(dt p) f -> p dt f", p=128))
        if sc == 0:
            load_w_qk()
            dma_x_chunk(xk_all, xkT, 0)
        else:
            # stream xk chunks between xv chunks; xq afterwards
            dma_x_chunk(xk_all, xkT, sc)
        for sti in range(ST_PER_SC):
            ps = pmm_ps.tile([128, GF], f32, tag="mm")
            col = sti * 128
            for dt in range(DT_N):
                nc.tensor.matmul(
                    ps[:], xv_all[:, dt * SC_W + col:dt * SC_W + col + 128],
                    wv_sb[dt], start=(dt == 0), stop=(dt == DT_N - 1))
            t = pv.tile([128, G * 65], bf16, tag="v")
            tv = t[:].rearrange("p (g e) -> p g e", e=65)
            nc.vector.tensor_add(
                tv[:, :, 0:64], ps[:].rearrange("p (g d) -> p g d", d=64),
                bvb[:].rearrange("p (g d) -> p g d", d=64))
            nc.vector.memset(tv[:, :, 64:65], 1.0)
            v_sb.append(t)
    for sc in range(SC_N):
        dma_x_chunk(xq_all, xqT, sc)
    x_sb = {"k": [xk_all[:, dt * S:(dt + 1) * S] for dt in range(DT_N)],
            "q": [xq_all[:, dt * S:(dt + 1) * S] for dt in range(DT_N)]}
    load_wo()

    qt_sb = [pqt.tile([128, S], bf16, tag="q", name=f"qt{i}") for i in range(FT_N)]
    kt_sb = [pkt.tile([128, S], bf16, tag="k", name=f"kt{i}") for i in range(FT_N)]

    def proj_group(name, wlist, bkey, dst, ft, sc):
        out_t = dst[ft]
        ps = pmm_ps.tile([128, SC_W], f32, tag="mm")
        for dt in range(DT_N):
            nc.tensor.matmul(
                ps[:], wlist[dt][:, ft * 128:(ft + 1) * 128],
                x_sb[name][dt][:, sc * SC_W:(sc + 1) * SC_W],
                start=(dt == 0), stop=(dt == DT_N - 1))
        nc.vector.tensor_scalar_add(
            out_t[:, sc * SC_W:(sc + 1) * SC_W], ps[:], holder[bkey][ft])

    # first attention block's direct inputs, inline
    proj_group("k", wk_sb, "bk", kt_sb, 0, 0)
    proj_group("q", wq_sb, "bq", qt_sb, 0, 0)

    # ---- deadline-tracked filler. Keys: ("K", ft, sc) must be emitted
    # before attention block (qc, ft) reads kt[ft][:, sc*SC_W...] at
    # k-step ST_PER_SC*sc; ("Q", ft, qc) before block (qc, ft) k-step 0;
    # ("O", ...) (out-projections) any time. Popping early is safe (WAR
    # deps are tracked); popping late would be a race, hence the due-check
    # before every k-step. ----
    def k_item(ft, sc):
        return ("K", ft, sc), lambda: proj_group("k", wk_sb, "bk", kt_sb, ft, sc)

    def q_item(ft, sc):
        return ("Q", ft, sc), lambda: proj_group("q", wq_sb, "bq", qt_sb, ft, sc)

    pe_filler = deque()
    for sc in range(1, SC_N):
        pe_filler.append(k_item(0, sc))
    for ft in range(1, FT_N):
        pe_filler.append(k_item(ft, 0))
        pe_filler.append(q_item(ft, 0))
        for sc in range(1, SC_N):
            pe_filler.append(k_item(ft, sc))
    for sc in range(1, SC_N):
        for ft in range(FT_N):
            pe_filler.append(q_item(ft, sc))

    def due(key, qc, ft, kt):
        kind, f, s = key
        if kind == "K":
            return f == ft and s <= kt // ST_PER_SC
        if kind == "Q":
            return s < qc or (s == qc and f <= ft)
        return False

    def cadence(qc, ft, kt):
        if qc == 0 and ft == 0:
            return kt >= 6 and kt % 3 == 0
        if qc == 0:
            return kt % 3 == 0
        return kt % 4 == 0

    ctx_sb = [None] * FT_N

    def ctx_mm(ctx_h, ft, kt, pt):
        for h in range(2):
            h65 = (2 * ft + h) * 65
            nc.tensor.matmul(
                ctx_h[h][:, :], v_sb[kt][:, h65:h65 + 65],
                pt[:, h * QC:(h + 1) * QC],
                start=(kt == 0), stop=(kt == KT_N - 1),
                skip_group_check=True)

    def outproj_group(qc, su):
        rows = slice(qc * QC + su * 128, qc * QC + (su + 1) * 128)
        o = pout.tile([128, D], bf16, tag="o")
        for j in range(J_N):
            ps = pmm_ps.tile([128, JW], f32, tag="mm")
            for ft in range(FT_N):
                nc.tensor.matmul(
                    ps[:], ctx_sb[ft][:, rows], wo_sb[ft][:, j * JW:(j + 1) * JW],
                    start=(ft == 0), stop=(ft == FT_N - 1))
            nc.vector.tensor_copy(o[:, j * JW:(j + 1) * JW], ps[:])
        nc.sync.dma_start(out_d[rows, :], o[:])

    # ---- attention, q-chunk major; filler interleaved between k-steps ----
    for qc in range(QC_N):
        qs = slice(qc * QC, (qc + 1) * QC)
        for ft in range(FT_N):
            if ctx_sb[ft] is None:
                ctx_sb[ft] = pctx.tile([128, S], bf16, tag="ctx", name=f"ctxsb{ft}")
            ctx_h = [pctx_ps.tile([65, QC], f32, tag="ctx", name=f"ctxps{h}")
                     for h in range(2)]
            pt_tiles = []
            for kt in range(KT_N):
                while pe_filler and due(pe_filler[0][0], qc, ft, kt):
                    pe_filler.popleft()[1]()
                if pe_filler and cadence(qc, ft, kt):
                    pe_filler.popleft()[1]()
                st_ps = pst_ps.tile([128, 2 * QC], f32, tag="st")
                ks = slice(kt * 128, (kt + 1) * 128)
                for h in range(2):
                    hp = slice(h * 64, (h + 1) * 64)
                    nc.tensor.matmul(
                        st_ps[:, h * QC:(h + 1) * QC],
                        kt_sb[ft][hp, ks], qt_sb[ft][hp, qs],
                        start=True, stop=True, tile_position=(h * 64, 0))
                pt = ppt.tile([128, 2 * QC], bf16, tag="pt")
                nc.scalar.activation(pt[:], st_ps[:], EXP, scale=0.125)
                pt_tiles.append(pt)
                # ctx matmuls run one k-step behind the scores so the PE
                # never queues a PT-dependent matmul ahead of the next
                # scores pair (keeps ACT fed back-to-back)
                if kt >= 1:
                    ctx_mm(ctx_h, ft, kt - 1, pt_tiles[kt - 1])
            ctx_mm(ctx_h, ft, KT_N - 1, pt_tiles[KT_N - 1])
            # normalize: evict both heads' raw ctx+rowsum to one bf16 SBUF
            # tile (frees the PSUM banks), recip row 64 with a 64-down
            # partition shift to p0, one partition broadcast, two 2x-mode
            # muls (odd head written with a +64 output partition shift).
            ctxr = psm.tile([65, 2 * QC], bf16, tag="ctxr", bufs=2)
            for h in range(2):
                nc.vector.tensor_copy(
                    ctxr[:, h * QC:(h + 1) * QC], ctx_h[h][:, :])
            invc = psm.tile([1, 2 * QC], bf16, tag="invc", bufs=2)
            with nc.allow_low_precision(reason="bf16 softmax inv rowsum"):
                nc.vector.reciprocal(invc[0:1, :], ctxr[64:65, :])
            bc = psm.tile([128, 2 * QC], bf16, tag="bc", bufs=2)
            nc.gpsimd.partition_broadcast(bc[:, :], invc[0:1, :])
            nc.vector.tensor_mul(
                ctx_sb[ft][0:64, qs], ctxr[0:64, 0:QC], bc[0:64, 0:QC])
            nc.vector.tensor_mul(
                ctx_sb[ft][64:128, qs], ctxr[0:64, QC:2 * QC], bc[0:64, QC:2 * QC])
        # queue this q-chunk's out-projection as filler for later blocks
        for su in range(SUB):
            pe_filler.append(
                (("O", qc, su), lambda qc=qc, su=su: outproj_group(qc, su)))
    while pe_filler:
        pe_filler.popleft()[1]()
    return nc


def build_nc(S=None, D=None, G=None, QC=None, num_devices=N_CORES, repeat=1):
    cfg = dict(FULL)
    for k, v in (("S", S), ("D", D), ("G", G), ("QC", QC)):
        if v is not None:
            cfg[k] = v
    from concourse import bacc
    nc = bacc.Bacc("TRN2", target_bir_lowering=False, debug=False,
                   num_devices=num_devices)
    build_body(nc, **cfg, repeat=repeat)
    nc.compile()
    return nc


def shard_inputs(q, k, v, Wq, bq, Wk, bk, Wv, bv, Wo, bo,
                 S=None, D=None, G=None, n_cores=N_CORES):
    """Build the per-core input maps (host-side shard + transpose + bf16 cast)."""
    S = S or FULL["S"]
    D = D or FULL["D"]
    G = G or FULL["G"]
    GF = G * DH
    FT_N = GF // 128
    n_groups = Wq.shape[0] // GF
    in_maps = []
    for c in range(n_cores):
        b, g = divmod(c, n_groups)
        gs = slice(g * GF, (g + 1) * GF)
        m = {
            "xqT": np.ascontiguousarray(q[b].T).astype(BF16),
            "xkT": np.ascontiguousarray(k[b].T).astype(BF16),
            "xvT": np.ascontiguousarray(v[b].T).astype(BF16),
            "wqT": np.ascontiguousarray(Wq[gs, :].T).astype(BF16),
            "wkT": np.ascontiguousarray(Wk[gs, :].T).astype(BF16),
            "wvT": np.ascontiguousarray(Wv[gs, :].T).astype(BF16),
            "woT": np.ascontiguousarray(Wo[:, gs].T).astype(BF16),
            "bq": np.ascontiguousarray(bq[gs]).reshape(FT_N, 128).astype(np.float32),
            "bk": np.ascontiguousarray(bk[gs]).reshape(FT_N, 128).astype(np.float32),
            "bv": np.ascontiguousarray(bv[gs]).reshape(1, GF).astype(np.float32),
        }
        in_maps.append(m)
    return in_maps


def gather_outputs(results, bo, n_groups=2):
    """Sum head-group partials per batch and add the output bias."""
    n_b = len(results) // n_groups
    outs = []
    for b in range(n_b):
        acc = results[b * n_groups]["out"].astype(np.float32)
        for g in range(1, n_groups):
            acc = acc + results[b * n_groups + g]["out"].astype(np.float32)
        outs.append(acc + np.asarray(bo, np.float32)[None, :])
    return np.stack(outs, axis=0)


_NC_CACHE = {}


def kernel(q, k, v, Wq, bq, Wk, bk, Wv, bv, Wo, bo):
    from concourse.bass_utils import run_bass_kernel_spmd
    key = "full"
    if key not in _NC_CACHE:
        _NC_CACHE[key] = build_nc()
    nc = _NC_CACHE[key]
    in_maps = shard_inputs(q, k, v, Wq, bq, Wk, bk, Wv, bv, Wo, bo)
    res = run_bass_kernel_spmd(nc, in_maps, core_ids=list(range(N_CORES)))
    return gather_outputs(res.results, bo)
